# revision 1
# baseline (speedup 1.0000x reference)
"""Trainium2 Bass kernel for nn_MultiHeadAttention (x:[2,2048,512], 8 heads, d=64).

Sharding: 8 cores = 2 batches x 4 head-pairs. Each core computes the QKV
projection for its 2 heads, the attention, and a partial (row-split) O
projection. Host sums the 4 partials per batch and adds the output bias.

Per-core on-device layout (all contractions land on SBUF partitions):
  xT  [512, 2048]  = x[b].T                (host pre-transposed)
  QT  [128, 2048]  = wq.T @ xT             (head dims 2x64 on partitions)
  KT  [128, 2048]  = wk.T @ xT
  V   [2048, 128]  = xT.T @ wv             (natural; k-seq on partitions)
  PT  [2048, q]    = exp(scale * KT_h.T @ QT_h)   (scores^T, per head, bf16)
  avs [128, q]     = [V_h0|V_h1].T @ [PT_h0|PT_h1]  (one PSUM bank, both heads)
  sums[128, q]     = ones64.T @ PT_h  (denominators, PE-replicated per head half)
  Z   [128, 2048]  = avs * recip(sums)              (attn out, heads-transposed)
  out [2048, 512]  = Z.T @ wo                       (partial; host reduces)

PSUM accumulation-group trick: a bank supports one start/stop group, so each
shared bank is opened by an N=1 zero-weight "clear" matmul (start=True) and
closed by another (stop=True); all real matmuls accumulate with start=False
in any schedule order (per-element has_written handles first-write).
"""

import sys

import numpy as np

for _p in ("/opt/trn_rl_repo",):
    if _p not in sys.path:
        sys.path.insert(0, _p)

import concourse.bass as bass  # noqa: E402
import concourse.tile as tile  # noqa: E402
from concourse import bacc, mybir  # noqa: E402
from concourse.bass_utils import run_bass_kernel_spmd  # noqa: E402

EMBED = 512
NH = 8
HD = 64
S = 2048
B = 2
SCALE = HD ** -0.5
F32 = mybir.dt.float32

# float32r: single-pass matmul (tf32-like); cannot target dst partition != 0,
# so the AV/sums matmuls (which write to partition 64) use bf16 inputs.
MM_DT = mybir.dt.float32r
AV_DT = mybir.dt.bfloat16

N_KT = EMBED // 128   # 4 contraction k-tiles for the projections
N_QT = S // 512       # 4 q column tiles
N_ST = S // 128       # 16 seq tiles of 128


def build_nc():
    nc = bacc.Bacc("TRN2", target_bir_lowering=False, debug=False)

    xT_d = nc.dram_tensor("xT", [EMBED, S], MM_DT, kind="ExternalInput").ap()
    wq_d = nc.dram_tensor("wq", [EMBED, 128], MM_DT, kind="ExternalInput").ap()
    wk_d = nc.dram_tensor("wk", [EMBED, 128], MM_DT, kind="ExternalInput").ap()
    wv_d = nc.dram_tensor("wv", [EMBED, 128], MM_DT, kind="ExternalInput").ap()
    wo_d = nc.dram_tensor("wo", [128, EMBED], MM_DT, kind="ExternalInput").ap()
    out_d = nc.dram_tensor("out", [S, EMBED], F32, kind="ExternalOutput").ap()

    with tile.TileContext(nc) as tc:
        with (
            tc.tile_pool(name="persist", bufs=1) as persist,
            tc.tile_pool(name="pt_pool", bufs=4) as pt_pool,
            tc.tile_pool(name="norm", bufs=2) as norm_pool,
            tc.tile_pool(name="ostage", bufs=3) as ostage,
            tc.tile_pool(name="ps", bufs=2, space="PSUM") as ps_pool,
            tc.tile_pool(name="ps_avh0", bufs=2, space="PSUM") as ps_avh0_pool,
            tc.tile_pool(name="ps_avh1", bufs=1, space="PSUM") as ps_avh1_pool,
            tc.tile_pool(name="ps_po", bufs=1, space="PSUM") as ps_po_pool,
        ):
            # ---- load phase: few, large DMA descriptors (issue cost is
            # ~0.6us each); weights first so the first projection matmuls
            # only wait on the first xT tiles ----
            wq_sb = persist.tile([128, N_KT, 128], MM_DT)
            wk_sb = persist.tile([128, N_KT, 128], MM_DT)
            wv_sb = persist.tile([128, N_KT, 128], MM_DT)
            for w_sb, w_d in ((wk_sb, wk_d), (wq_sb, wq_d), (wv_sb, wv_d)):
                # one 3D descriptor: [part, ktile, col] <- [512, 128] dram
                nc.sync.dma_start(
                    out=w_sb,
                    in_=w_d.rearrange("(t p) m -> p t m", p=128),
                )
            wo_sb = persist.tile([128, EMBED], MM_DT)
            nc.sync.dma_start(out=wo_sb, in_=wo_d)
            xT_sb = persist.tile([128, N_KT, S], MM_DT)  # [part, ktile, seq]
            xT_r = xT_d.rearrange("(t p) s -> p t s", p=128)
            half = S // 2
            for k in range(N_KT):
                nc.sync.dma_start(
                    out=xT_sb[:, k, 0:half], in_=xT_r[:, k, 0:half]
                )
                nc.scalar.dma_start(
                    out=xT_sb[:, k, half:S], in_=xT_r[:, k, half:S]
                )


            # ---- qkv projections ----
            KT_sb = persist.tile([128, S], MM_DT)
            QT_sb = persist.tile([128, S], MM_DT)
            for qt in range(N_QT):
                for w_sb, t_sb in ((wk_sb, KT_sb), (wq_sb, QT_sb)):
                    qs = bass.ts(qt, 512)
                    ps = ps_pool.tile([128, 2, 512], F32, tag="ps")
                    for k in range(N_KT):
                        nc.tensor.matmul(
                            ps[:, 0, :],
                            w_sb[:, k, :],
                            xT_sb[:, k, qs],
                            start=(k == 0),
                            stop=(k == N_KT - 1),
                        )
                    nc.vector.tensor_copy(t_sb[:, qs], ps[:, 0, :])
            # V with a baked all-ones 65th column: the M=65 AV matmuls then
            # produce the softmax denominators in psum row 64 for free
            V_sb = persist.tile([128, N_ST, 2, HD + 1], AV_DT)
            nc.vector.memset(V_sb, 1.0)
            for st in range(N_ST):
                ps = ps_pool.tile([128, 2, 512], F32, tag="ps")
                for k in range(N_KT):
                    nc.tensor.matmul(
                        ps[:, 0, 0:128],
                        xT_sb[:, k, bass.ts(st, 128)],
                        wv_sb[:, k, :],
                        start=(k == 0),
                        stop=(k == N_KT - 1),
                    )
                nc.vector.tensor_copy(V_sb[:, st, 0, 0:HD], ps[:, 0, 0:64])
                nc.vector.tensor_copy(V_sb[:, st, 1, 0:HD], ps[:, 0, 64:128])

            # ---- attention (+ interleaved O-projection of the previous tile) ----
            Z_sb = persist.tile([128, S], MM_DT)  # normalized attn out^T, 2 heads

            def emit_oproj_one(src_qt, mi):
                # one O-projection matmul for q range src_qt, emitted inside
                # the next tile's k-loop; single dedicated psum bank reused
                # serially (the staging copy from 2 chunks earlier is done)
                m = 4 * src_qt + mi
                po = ps_po_pool.tile([128, 512], F32, tag="po")
                nc.tensor.matmul(
                    po, Z_sb[:, bass.ts(m, 128)], wo_sb, start=True, stop=True,
                )
                ot = ostage.tile([128, 512], F32, tag="ot")
                nc.vector.tensor_copy(ot, po)
                nc.sync.dma_start(out=out_d[bass.ts(m, 128), :], in_=ot)

            for qt in range(N_QT):
                qs = bass.ts(qt, 512)
                av0 = ps_avh0_pool.tile([128, 512], F32, tag="avh0")
                av1 = ps_avh1_pool.tile([128, 512], F32, tag="avh1")
                for ks in range(N_ST):  # one 128-wide k block per chunk
                    if qt > 0 and ks in (4, 6, 8, 10):
                        emit_oproj_one(qt - 1, (ks - 4) // 2)
                    kk = bass.ts(ks, 128)
                    s = ps_pool.tile([128, 2, 512], F32, tag="ps")
                    # adjacent head-paired score matmuls: disjoint row groups
                    # (0:64 / 64:128) execute concurrently on the PE array
                    nc.tensor.matmul(
                        s[:, 0, :], KT_sb[0:64, kk], QT_sb[0:64, qs],
                        start=True, stop=True,
                    )
                    nc.tensor.matmul(
                        s[:, 1, :], KT_sb[64:128, kk], QT_sb[64:128, qs],
                        start=True, stop=True,
                    )
                    pt = pt_pool.tile([128, 2, 512], AV_DT, tag="pt")
                    nc.scalar.activation(
                        out=pt, in_=s, func=mybir.ActivationFunctionType.Exp,
                        scale=SCALE,
                    )
                    # M=65: rows 0:64 accumulate V^T @ PT, row 64 (ones
                    # column) accumulates the softmax denominators
                    nc.tensor.matmul(
                        av0[0:HD + 1, :], V_sb[:, ks, 0, :], pt[:, 0, :],
                        start=(ks == 0), stop=(ks == N_ST - 1),
                    )
                    nc.tensor.matmul(
                        av1[0:HD + 1, :], V_sb[:, ks, 1, :], pt[:, 1, :],
                        start=(ks == 0), stop=(ks == N_ST - 1),
                    )
                # stage the accumulators out of psum immediately so the next
                # tile's AV matmuls are never blocked by the normalize chain
                avc_sb = norm_pool.tile([64, 2, 512], F32, tag="avc")
                nc.vector.tensor_copy(avc_sb[:, 0, :], av0[0:64, :])
                nc.vector.tensor_copy(avc_sb[:, 1, :], av1[0:64, :])
                s_row = norm_pool.tile([1, 2, 512], F32, tag="s_row")
                nc.vector.tensor_copy(s_row[0:1, 0, :], av0[64:65, :])
                nc.vector.tensor_copy(s_row[0:1, 1, :], av1[64:65, :])
                r0_sb = norm_pool.tile([1, 2, 512], F32, tag="r0")
                nc.vector.reciprocal_approx_fast(
                    out=r0_sb[0:1, 0, :], in_=s_row[0:1, 0, :]
                )
                nc.vector.reciprocal_approx_fast(
                    out=r0_sb[0:1, 1, :], in_=s_row[0:1, 1, :]
                )
                rb_sb = norm_pool.tile([64, 2, 512], F32, tag="rb")
                nc.gpsimd.partition_broadcast(
                    out_ap=rb_sb[0:64, 0, :], in_ap=r0_sb[0:1, 0, :]
                )
                nc.gpsimd.partition_broadcast(
                    out_ap=rb_sb[0:64, 1, :], in_ap=r0_sb[0:1, 1, :]
                )
                nc.vector.tensor_mul(
                    Z_sb[0:64, qs], avc_sb[:, 0, :], rb_sb[0:64, 0, :]
                )
                nc.vector.tensor_mul(
                    Z_sb[64:128, qs], avc_sb[:, 1, :], rb_sb[0:64, 1, :]
                )
            for mi in range(4):
                emit_oproj_one(N_QT - 1, mi)

    nc.compile()
    return nc


_NC = None


def _get_nc():
    global _NC
    if _NC is None:
        _NC = build_nc()
    return _NC


def make_in_maps(x, w_qkv, w_o):
    x = np.ascontiguousarray(np.asarray(x, dtype=np.float32))
    w_qkv = np.asarray(w_qkv, dtype=np.float32)
    w_o = np.asarray(w_o, dtype=np.float32)
    in_maps = []
    xTs = [np.ascontiguousarray(x[b].T) for b in range(B)]
    for c in range(8):
        b, g = c // 4, c % 4
        cols = slice(2 * g * HD, (2 * g + 2) * HD)
        in_maps.append({
            "xT": xTs[b],
            "wq": np.ascontiguousarray(w_qkv[:, :EMBED][:, cols]),
            "wk": np.ascontiguousarray(w_qkv[:, EMBED:2 * EMBED][:, cols]),
            "wv": np.ascontiguousarray(w_qkv[:, 2 * EMBED:][:, cols]),
            "wo": np.ascontiguousarray(w_o[cols, :]),
        })
    return in_maps


def combine(results, b_o):
    partials = np.stack([r["out"] for r in results])  # [8, S, EMBED]
    out = partials.reshape(B, 4, S, EMBED).sum(axis=1)
    return (out + np.asarray(b_o, dtype=np.float32)).astype(np.float32)


def kernel(x, w_qkv, w_o, b_o):
    nc = _get_nc()
    res = run_bass_kernel_spmd(nc, make_in_maps(x, w_qkv, w_o), core_ids=list(range(8)))
    return combine(res.results, b_o)



# revision 3
# speedup vs baseline: 1.2241x; 1.2241x over previous
"""Trainium2 Bass kernel for nn_MultiHeadAttention (x:[2,2048,512], 8 heads, d=64).

Sharding: 8 cores = 2 batches x 4 head-pairs. Each core computes the QKV
projection for its 2 heads, the attention, and a partial (row-split) O
projection. Host sums the 4 partials per batch and adds the output bias.

All matmul inputs are bf16 (validated ~5e-3 max rel err vs the 2e-2 gate);
accumulation is always fp32 in PSUM. Host pre-arranges every input so each
DMA descriptor row is contiguous (xT as [128, 4k, 2048], weights as
[128, 4k, 128]).

Per-core schedule (single Act-engine EXP stream is the bound, ~69us):
  t0      gpsimd memsets; DMA issues (weights on sync, xT k-tiles on
          scalar/vector queues); PE warm-up filler matmuls (p-state ramp)
  phase A k-major KQ projection: as xT k-tile k lands, accumulate
          KT qt0-3 (ps pool banks) + QT qt0/qt1 (av banks) with
          start=(k==0)/stop=(k==3); DVE casts psum->sbuf bf16
  V proj  bf16 (no fp32r N<256 penalty): blocks of 4 seq-tiles share one
          PSUM bank (sequential per-region accumulation groups); groups
          0/1 before attention, 2/3 + QT qt2/qt3 interleaved into the
          chunk stream
  chunks  64 chunks (qt,ks): scores emitted ONE chunk ahead so the next
          EXP never waits on the in-order PE queue; EXP on Act (psum fp32
          -> sbuf bf16, scale fused); AV with M=65 ones-column trick
          accumulating softmax denominators in psum row 64
  norm    per qt: DVE stage + reciprocal, gpsimd partition broadcast,
          DVE multiply -> Z; O-projection of qt-1 interleaved into qt's
          chunks, staged and DMA'd out on sync
"""

import sys

import numpy as np

for _p in ("/opt/trn_rl_repo",):
    if _p not in sys.path:
        sys.path.insert(0, _p)

import ml_dtypes  # noqa: E402

import concourse.bass as bass  # noqa: E402
import concourse.tile as tile  # noqa: E402
from concourse import bacc, mybir  # noqa: E402
from concourse.bass_utils import run_bass_kernel_spmd  # noqa: E402

EMBED = 512
NH = 8
HD = 64
S = 2048
B = 2
SCALE = HD ** -0.5
F32 = mybir.dt.float32
F32R = mybir.dt.float32r
BF16 = mybir.dt.bfloat16

N_KT = EMBED // 128   # 4 contraction k-tiles for the projections
N_QT = S // 512       # 4 q column tiles
N_ST = S // 128       # 16 seq tiles of 128

N_FILL = 12           # PE warm-up matmuls during the DMA phase


def build_nc():
    nc = bacc.Bacc("TRN2", target_bir_lowering=False, debug=False)

    xTb_d = nc.dram_tensor("xTb", [128, N_KT, S], BF16, kind="ExternalInput").ap()
    wqb_d = nc.dram_tensor("wqb", [128, N_KT, 128], BF16, kind="ExternalInput").ap()
    wkb_d = nc.dram_tensor("wkb", [128, N_KT, 128], BF16, kind="ExternalInput").ap()
    wvb_d = nc.dram_tensor("wvb", [128, N_KT, 128], BF16, kind="ExternalInput").ap()
    wo_d = nc.dram_tensor("wo", [128, EMBED], F32R, kind="ExternalInput").ap()
    out_d = nc.dram_tensor("out", [S, EMBED], F32, kind="ExternalOutput").ap()

    with tile.TileContext(nc) as tc:
        with (
            tc.tile_pool(name="persist", bufs=1) as persist,
            tc.tile_pool(name="pt_pool", bufs=6) as pt_pool,
            tc.tile_pool(name="norm", bufs=2) as norm_pool,
            tc.tile_pool(name="ostage", bufs=3) as ostage,
            tc.tile_pool(name="ps", bufs=2, space="PSUM") as ps_pool,
            tc.tile_pool(name="ps_a0", bufs=1, space="PSUM") as a0_pool,
            tc.tile_pool(name="ps_a1", bufs=1, space="PSUM") as a1_pool,
            tc.tile_pool(name="ps_po", bufs=1, space="PSUM") as po_pool,
            tc.tile_pool(name="ps_tr", bufs=1, space="PSUM") as tr_pool,
        ):
            # ---- t0: small sbuf constants on gpsimd (keeps DVE free) ----
            dummy = persist.tile([128, 512], BF16)
            nc.gpsimd.memset(dummy, 0.0)
            # V with a baked all-ones 65th column: the M=65 AV matmuls then
            # produce the softmax denominators in psum row 64 for free
            V_sb = persist.tile([128, N_ST, 2, HD + 1], BF16)
            nc.gpsimd.memset(V_sb, 1.0)

            # ---- DMA issues: weights on sync; xT k-tiles split over the
            # scalar and vector queues so k0/k1 stream while k2/k3 follow ----
            wkb_sb = persist.tile([128, N_KT, 128], BF16)
            wqb_sb = persist.tile([128, N_KT, 128], BF16)
            wvb_sb = persist.tile([128, N_KT, 128], BF16)
            wo_sb = persist.tile([128, EMBED], F32R)
            xTb_sb = persist.tile([128, N_KT, S], BF16)
            nc.sync.dma_start(out=wkb_sb, in_=wkb_d)
            nc.sync.dma_start(out=wqb_sb, in_=wqb_d)
            nc.sync.dma_start(out=wvb_sb, in_=wvb_d)
            nc.sync.dma_start(out=wo_sb, in_=wo_d)
            for k, eng in ((0, nc.scalar), (1, nc.gpsimd), (2, nc.scalar), (3, nc.gpsimd)):
                eng.dma_start(out=xTb_sb[:, k, :], in_=xTb_d[:, k, :])

            # ---- PE warm-up fillers: keep the tensor engine continuously
            # busy through the DMA window so the p-state ramp (0.65 -> 2.4
            # GHz after 3us continuous) completes before real work ----
            fill_pools = (a0_pool, a1_pool, po_pool, tr_pool)
            for i in range(N_FILL):
                fl = fill_pools[i % 4].tile([128, 512], F32, tag="b", name="fl")
                nc.tensor.matmul(fl, dummy[:, 0:128], dummy, start=True, stop=True)

            # ---- phase A: k-major KQ projection, accumulating in psum as
            # each xT k-tile lands. KT qt0-3 in the two ps-pool tiles,
            # QT qt0/qt1 in the av banks (free until attention starts) ----
            KTps = [ps_pool.tile([128, 2, 512], F32, tag="ps", name="KTps") for _ in range(2)]
            QT0ps = a0_pool.tile([128, 512], F32, tag="b", name="QT0ps")
            QT1ps = a1_pool.tile([128, 512], F32, tag="b", name="QT1ps")
            for k in range(N_KT):
                st, sp = k == 0, k == N_KT - 1
                for qt in range(N_QT):
                    nc.tensor.matmul(
                        KTps[qt // 2][:, qt % 2, :],
                        wkb_sb[:, k, :], xTb_sb[:, k, bass.ts(qt, 512)],
                        start=st, stop=sp,
                    )
                nc.tensor.matmul(
                    QT0ps, wqb_sb[:, k, :], xTb_sb[:, k, 0:512], start=st, stop=sp,
                )
                nc.tensor.matmul(
                    QT1ps, wqb_sb[:, k, :], xTb_sb[:, k, 512:1024], start=st, stop=sp,
                )
            # casts psum -> sbuf bf16; KT0/QT0/QT1 first (they gate chunk 0)
            KTb = persist.tile([128, S], BF16)
            QTb = persist.tile([128, S], BF16)
            nc.vector.tensor_copy(KTb[:, 0:512], KTps[0][:, 0, :])
            nc.vector.tensor_copy(QTb[:, 0:512], QT0ps)
            nc.vector.tensor_copy(QTb[:, 512:1024], QT1ps)
            nc.vector.tensor_copy(KTb[:, 512:1024], KTps[0][:, 1, :])
            nc.vector.tensor_copy(KTb[:, 1024:1536], KTps[1][:, 0, :])
            nc.vector.tensor_copy(KTb[:, 1536:2048], KTps[1][:, 1, :])

            # ---- V projection groups: 4 seq-blocks [128,128] side by side
            # in one psum bank. Per-region accumulation groups run
            # sequentially on the in-order PE, so plain start/stop per
            # block is safe; per-block casts feed AV as early as possible.
            def emit_vgroup(pool, g):
                vg = pool.tile([128, 512], F32, tag="b", name="vg")
                for b in range(4):
                    st = 4 * g + b
                    for k in range(N_KT):
                        nc.tensor.matmul(
                            vg[:, bass.ts(b, 128)],
                            xTb_sb[:, k, bass.ts(st, 128)], wvb_sb[:, k, :],
                            start=(k == 0), stop=(k == N_KT - 1),
                        )
                    nc.vector.tensor_copy(
                        V_sb[:, st, :, 0:HD],
                        vg.rearrange("p (b h d) -> p b h d", b=4, h=2)[:, b],
                    )

            emit_vgroup(po_pool, 0)
            emit_vgroup(tr_pool, 1)

            # ---- attention chunk stream ----
            Z_sb = persist.tile([128, S], F32R)  # normalized attn out^T, 2 heads

            def emit_scores(n):
                qt, ks = n // N_ST, n % N_ST
                s = ps_pool.tile([128, 2, 512], F32, tag="ps", name="s")
                kk = bass.ts(ks, 128)
                qs = bass.ts(qt, 512)
                nc.tensor.matmul(
                    s[:, 0, :], KTb[0:64, kk], QTb[0:64, qs], start=True, stop=True,
                )
                nc.tensor.matmul(
                    s[:, 1, :], KTb[64:128, kk], QTb[64:128, qs], start=True, stop=True,
                )
                return s

            def emit_oproj_one(src_qt, mi):
                m = 4 * src_qt + mi
                po = po_pool.tile([128, 512], F32, tag="b", name="po")
                nc.tensor.matmul(
                    po, Z_sb[:, bass.ts(m, 128)], wo_sb, start=True, stop=True,
                )
                ot = ostage.tile([128, 512], F32, tag="ot")
                nc.vector.tensor_copy(ot, po)
                nc.sync.dma_start(out=out_d[bass.ts(m, 128), :], in_=ot)

            def emit_qt_extras(qt, ks):
                # deferred projection work threaded into the chunk stream
                if qt == 0 and ks == 2:
                    emit_vgroup(po_pool, 2)
                if qt == 0 and ks == 6:
                    emit_vgroup(tr_pool, 3)
                if qt == 1 and ks == 1:
                    q2 = po_pool.tile([128, 512], F32, tag="b", name="q2")
                    for k in range(N_KT):
                        nc.tensor.matmul(
                            q2, wqb_sb[:, k, :], xTb_sb[:, k, 1024:1536],
                            start=(k == 0), stop=(k == N_KT - 1),
                        )
                    nc.vector.tensor_copy(QTb[:, 1024:1536], q2)
                if qt == 1 and ks == 3:
                    q3 = tr_pool.tile([128, 512], F32, tag="b", name="q3")
                    for k in range(N_KT):
                        nc.tensor.matmul(
                            q3, wqb_sb[:, k, :], xTb_sb[:, k, 1536:2048],
                            start=(k == 0), stop=(k == N_KT - 1),
                        )
                    nc.vector.tensor_copy(QTb[:, 1536:2048], q3)
                if qt > 0 and ks in (4, 6, 8, 10):
                    emit_oproj_one(qt - 1, (ks - 4) // 2)

            s_cur = emit_scores(0)
            for qt in range(N_QT):
                qs = bass.ts(qt, 512)
                av0 = a0_pool.tile([128, 512], F32, tag="b", name="av0")
                av1 = a1_pool.tile([128, 512], F32, tag="b", name="av1")
                for ks in range(N_ST):
                    n = qt * N_ST + ks
                    emit_qt_extras(qt, ks)
                    s_next = emit_scores(n + 1) if n + 1 < N_QT * N_ST else None
                    pt = pt_pool.tile([128, 2, 512], BF16, tag="pt")
                    nc.scalar.activation(
                        out=pt, in_=s_cur, func=mybir.ActivationFunctionType.Exp,
                        scale=SCALE,
                    )
                    s_cur = s_next
                    # M=65: rows 0:64 accumulate V^T @ PT, row 64 (ones
                    # column) accumulates the softmax denominators
                    nc.tensor.matmul(
                        av0[0:HD + 1, :], V_sb[:, ks, 0, :], pt[:, 0, :],
                        start=(ks == 0), stop=(ks == N_ST - 1),
                    )
                    nc.tensor.matmul(
                        av1[0:HD + 1, :], V_sb[:, ks, 1, :], pt[:, 1, :],
                        start=(ks == 0), stop=(ks == N_ST - 1),
                    )
                # stage the accumulators out of psum immediately so the next
                # tile's AV matmuls are never blocked by the normalize chain
                avc_sb = norm_pool.tile([64, 2, 512], F32, tag="avc")
                nc.vector.tensor_copy(avc_sb[:, 0, :], av0[0:64, :])
                nc.vector.tensor_copy(avc_sb[:, 1, :], av1[0:64, :])
                s_row = norm_pool.tile([1, 2, 512], F32, tag="s_row")
                nc.vector.tensor_copy(s_row[0:1, 0, :], av0[64:65, :])
                nc.vector.tensor_copy(s_row[0:1, 1, :], av1[64:65, :])
                r0_sb = norm_pool.tile([1, 2, 512], F32, tag="r0")
                nc.vector.reciprocal_approx_fast(
                    out=r0_sb[0:1, 0, :], in_=s_row[0:1, 0, :]
                )
                nc.vector.reciprocal_approx_fast(
                    out=r0_sb[0:1, 1, :], in_=s_row[0:1, 1, :]
                )
                rb_sb = norm_pool.tile([64, 2, 512], F32, tag="rb")
                nc.gpsimd.partition_broadcast(
                    out_ap=rb_sb[0:64, 0, :], in_ap=r0_sb[0:1, 0, :]
                )
                nc.gpsimd.partition_broadcast(
                    out_ap=rb_sb[0:64, 1, :], in_ap=r0_sb[0:1, 1, :]
                )
                nc.vector.tensor_mul(
                    Z_sb[0:64, qs], avc_sb[:, 0, :], rb_sb[0:64, 0, :]
                )
                nc.vector.tensor_mul(
                    Z_sb[64:128, qs], avc_sb[:, 1, :], rb_sb[0:64, 1, :]
                )
            for mi in range(4):
                emit_oproj_one(N_QT - 1, mi)

    nc.compile()
    return nc


_NC = None


def _get_nc():
    global _NC
    if _NC is None:
        _NC = build_nc()
    return _NC


def _tiled(a):
    """[512, N] -> [128, 4, N] bf16 with contiguous per-partition rows."""
    n = a.shape[1]
    return np.ascontiguousarray(
        a.reshape(N_KT, 128, n).transpose(1, 0, 2)
    ).astype(ml_dtypes.bfloat16)


def make_in_maps(x, w_qkv, w_o):
    x = np.asarray(x, dtype=np.float32)
    w_qkv = np.asarray(w_qkv, dtype=np.float32)
    w_o = np.asarray(w_o, dtype=np.float32)
    in_maps = []
    xTs = [_tiled(np.ascontiguousarray(x[b].T)) for b in range(B)]
    for c in range(8):
        b, g = c // 4, c % 4
        cols = slice(2 * g * HD, (2 * g + 2) * HD)
        in_maps.append({
            "xTb": xTs[b],
            "wqb": _tiled(w_qkv[:, :EMBED][:, cols]),
            "wkb": _tiled(w_qkv[:, EMBED:2 * EMBED][:, cols]),
            "wvb": _tiled(w_qkv[:, 2 * EMBED:][:, cols]),
            "wo": np.ascontiguousarray(w_o[cols, :]),
        })
    return in_maps


def combine(results, b_o):
    partials = np.stack([r["out"] for r in results])  # [8, S, EMBED]
    out = partials.reshape(B, 4, S, EMBED).sum(axis=1)
    return (out + np.asarray(b_o, dtype=np.float32)).astype(np.float32)


def kernel(x, w_qkv, w_o, b_o):
    nc = _get_nc()
    res = run_bass_kernel_spmd(nc, make_in_maps(x, w_qkv, w_o), core_ids=list(range(8)))
    return combine(res.results, b_o)


# revision 4
# speedup vs baseline: 1.2968x; 1.0593x over previous
"""Trainium2 Bass kernel for nn_MultiHeadAttention (x:[2,2048,512], 8 heads, d=64).

Sharding: 8 cores = 2 batches x 4 head-pairs. Each core computes the QKV
projection for its 2 heads, the attention, and a partial (row-split) O
projection. Host sums the 4 partials per batch and adds the output bias.

All matmul inputs are bf16 (validated ~5e-3 max rel err vs the 2e-2 gate);
accumulation is always fp32 in PSUM. Host pre-arranges every input so each
DMA descriptor row is contiguous. Partial outputs return as bf16 (host
upcasts before the reduce).

Per-core schedule (single Act-engine EXP stream is the bound, ~69us):
  t0      DMA issues: weights + xT k0 on sync; k1..k3 guard-chained on
          gpsimd (a 1-elem read of the previous k-tile before each issue
          serializes the transfers so k0 lands first and the k-major
          projection pipelines with arrival); PE warm-up fillers
  phase A k-major KQ projection: as xT k-tile k lands, accumulate
          KT qt0-3 (ps pool banks) + QT qt0/qt1 (av banks) with
          start=(k==0)/stop=(k==3); DVE casts psum->sbuf bf16
  V proj  bf16 (no fp32r N<256 penalty): 4 seq-blocks share one PSUM bank
          (sequential per-region accumulation groups on the in-order PE);
          group 0 up front, groups 1-3 threaded one block per chunk
  chunks  64 chunks (qt,ks), per-chunk PE emission order: scores(n+1)
          FIRST (so the next EXP never waits on queued extras), then one
          piece of deferred work, then AV(n). EXP on Act (psum fp32 ->
          sbuf bf16, scale fused); AV with M=65 ones-column trick
          accumulating softmax denominators in psum row 64
  norm    per qt: DVE stage + reciprocal, gpsimd partition broadcast,
          DVE multiply -> Z; O-projection of qt-1 threaded into qt's
          chunks alternating the po/tr banks; tail normalize reordered
          (s_row+recip before avc) and O-proj pipelined across two banks
"""

import sys

import numpy as np

for _p in ("/opt/trn_rl_repo",):
    if _p not in sys.path:
        sys.path.insert(0, _p)

import ml_dtypes  # noqa: E402

import concourse.bass as bass  # noqa: E402
import concourse.tile as tile  # noqa: E402
from concourse import bacc, mybir  # noqa: E402
from concourse.bass_utils import run_bass_kernel_spmd  # noqa: E402

EMBED = 512
NH = 8
HD = 64
S = 2048
B = 2
SCALE = HD ** -0.5
F32 = mybir.dt.float32
F32R = mybir.dt.float32r
BF16 = mybir.dt.bfloat16

N_KT = EMBED // 128   # 4 contraction k-tiles for the projections
N_QT = S // 512       # 4 q column tiles
N_ST = S // 128       # 16 seq tiles of 128

N_FILL = 8            # PE warm-up matmuls during the DMA phase


def build_nc():
    nc = bacc.Bacc("TRN2", target_bir_lowering=False, debug=False)

    xTb_d = nc.dram_tensor("xTb", [128, N_KT, S], BF16, kind="ExternalInput").ap()
    wqb_d = nc.dram_tensor("wqb", [128, N_KT, 128], BF16, kind="ExternalInput").ap()
    wkb_d = nc.dram_tensor("wkb", [128, N_KT, 128], BF16, kind="ExternalInput").ap()
    wvb_d = nc.dram_tensor("wvb", [128, N_KT, 128], BF16, kind="ExternalInput").ap()
    wo_d = nc.dram_tensor("wo", [128, EMBED], F32R, kind="ExternalInput").ap()
    out_d = nc.dram_tensor("out", [S, EMBED], BF16, kind="ExternalOutput").ap()

    with tile.TileContext(nc) as tc:
        with (
            tc.tile_pool(name="persist", bufs=1) as persist,
            tc.tile_pool(name="pt_pool", bufs=6) as pt_pool,
            tc.tile_pool(name="norm", bufs=2) as norm_pool,
            tc.tile_pool(name="ostage", bufs=4) as ostage,
            tc.tile_pool(name="ps", bufs=2, space="PSUM") as ps_pool,
            tc.tile_pool(name="ps_a0", bufs=1, space="PSUM") as a0_pool,
            tc.tile_pool(name="ps_a1", bufs=1, space="PSUM") as a1_pool,
            tc.tile_pool(name="ps_po", bufs=1, space="PSUM") as po_pool,
            tc.tile_pool(name="ps_tr", bufs=1, space="PSUM") as tr_pool,
        ):
            # ---- DMA issues first. sync: small weights, then xT k0.
            # gpsimd: k1..k3, each issue preceded by a 1-elem read of the
            # previous k-tile so the transfers serialize (k0 lands ~2.5us
            # after issue instead of sharing bandwidth 4 ways). ----
            wkb_sb = persist.tile([128, N_KT, 128], BF16)
            wqb_sb = persist.tile([128, N_KT, 128], BF16)
            wvb_sb = persist.tile([128, N_KT, 128], BF16)
            wo_sb = persist.tile([128, EMBED], F32R)
            xTb_sb = persist.tile([128, N_KT, S], BF16)
            nc.sync.dma_start(out=wkb_sb, in_=wkb_d)
            nc.sync.dma_start(out=wqb_sb, in_=wqb_d)
            nc.sync.dma_start(out=wvb_sb, in_=wvb_d)
            nc.sync.dma_start(out=wo_sb, in_=wo_d)
            nc.sync.dma_start(out=xTb_sb[:, 0, :], in_=xTb_d[:, 0, :])

            dummy = persist.tile([128, 512], BF16)
            nc.gpsimd.memset(dummy, 0.0)
            guard = persist.tile([1, 4], F32)
            for k in range(1, N_KT):
                nc.gpsimd.tensor_scalar_add(
                    out=guard[0:1, k - 1:k], in0=xTb_sb[0:1, k - 1, 0:1], scalar1=0.0,
                )
                nc.gpsimd.dma_start(out=xTb_sb[:, k, :], in_=xTb_d[:, k, :])
            # V with a baked all-ones 65th column: the M=65 AV matmuls then
            # produce the softmax denominators in psum row 64 for free
            V_sb = persist.tile([128, N_ST, 2, HD + 1], BF16)
            nc.gpsimd.memset(V_sb, 1.0)

            # ---- PE warm-up fillers: keep the tensor engine busy through
            # the DMA window so the p-state ramp (0.65 -> 2.4 GHz after 3us
            # continuous) completes before real work ----
            fill_pools = (a0_pool, a1_pool, po_pool, tr_pool)
            for i in range(N_FILL):
                fl = fill_pools[i % 4].tile([128, 512], F32, tag="b", name="fl")
                nc.tensor.matmul(fl, dummy[:, 0:128], dummy, start=True, stop=True)

            # ---- phase A: k-major KQ projection, accumulating in psum as
            # each xT k-tile lands. KT qt0-3 in the two ps-pool tiles,
            # QT qt0/qt1 in the av banks (free until attention starts) ----
            KTps = [ps_pool.tile([128, 2, 512], F32, tag="ps", name="KTps") for _ in range(2)]
            QT0ps = a0_pool.tile([128, 512], F32, tag="b", name="QT0ps")
            QT1ps = a1_pool.tile([128, 512], F32, tag="b", name="QT1ps")
            for k in range(N_KT):
                st, sp = k == 0, k == N_KT - 1
                for qt in range(N_QT):
                    nc.tensor.matmul(
                        KTps[qt // 2][:, qt % 2, :],
                        wkb_sb[:, k, :], xTb_sb[:, k, bass.ts(qt, 512)],
                        start=st, stop=sp,
                    )
                nc.tensor.matmul(
                    QT0ps, wqb_sb[:, k, :], xTb_sb[:, k, 0:512], start=st, stop=sp,
                )
                nc.tensor.matmul(
                    QT1ps, wqb_sb[:, k, :], xTb_sb[:, k, 512:1024], start=st, stop=sp,
                )
            # casts psum -> sbuf bf16; KT0/QT0/QT1 first (they gate chunk 0)
            KTb = persist.tile([128, S], BF16)
            QTb = persist.tile([128, S], BF16)
            nc.vector.tensor_copy(KTb[:, 0:512], KTps[0][:, 0, :])
            nc.vector.tensor_copy(QTb[:, 0:512], QT0ps)
            nc.vector.tensor_copy(QTb[:, 512:1024], QT1ps)
            nc.vector.tensor_copy(KTb[:, 512:1024], KTps[0][:, 1, :])
            nc.vector.tensor_copy(KTb[:, 1024:1536], KTps[1][:, 0, :])
            nc.vector.tensor_copy(KTb[:, 1536:2048], KTps[1][:, 1, :])

            # ---- V projection: 4 seq-blocks [128,128] side by side in one
            # psum bank; per-region accumulation groups run sequentially on
            # the in-order PE so plain start/stop per block is safe ----
            vg_state = {}

            def emit_vblock(pool, g, b):
                if b == 0:
                    vg_state[g] = pool.tile([128, 512], F32, tag="b", name="vg")
                vg = vg_state[g]
                stq = 4 * g + b
                for k in range(N_KT):
                    nc.tensor.matmul(
                        vg[:, bass.ts(b, 128)],
                        xTb_sb[:, k, bass.ts(stq, 128)], wvb_sb[:, k, :],
                        start=(k == 0), stop=(k == N_KT - 1),
                    )
                nc.vector.tensor_copy(
                    V_sb[:, stq, :, 0:HD],
                    vg.rearrange("p (b h d) -> p b h d", b=4, h=2)[:, b],
                )

            for b in range(4):
                emit_vblock(po_pool, 0, b)

            # ---- attention chunk stream ----
            Z_sb = persist.tile([128, S], F32R)  # normalized attn out^T, 2 heads

            def emit_scores(n):
                qt, ks = n // N_ST, n % N_ST
                s = ps_pool.tile([128, 2, 512], F32, tag="ps", name="s")
                kk = bass.ts(ks, 128)
                qs = bass.ts(qt, 512)
                nc.tensor.matmul(
                    s[:, 0, :], KTb[0:64, kk], QTb[0:64, qs], start=True, stop=True,
                )
                nc.tensor.matmul(
                    s[:, 1, :], KTb[64:128, kk], QTb[64:128, qs], start=True, stop=True,
                )
                return s

            def emit_oproj_piece(src_qt, pi, pool):
                # one N=256 half-column piece of the O-projection for
                # m-block m = pi // 2, embed cols (pi % 2) * 256
                m = 4 * src_qt + pi // 2
                cs = bass.ts(pi % 2, 256)
                po = pool.tile([128, 512], F32, tag="b", name="po")
                nc.tensor.matmul(
                    po[:, 0:256], Z_sb[:, bass.ts(m, 128)], wo_sb[:, cs],
                    start=True, stop=True,
                )
                ot = ostage.tile([128, 256], BF16, tag="ot")
                nc.vector.tensor_copy(ot, po[:, 0:256])
                nc.sync.dma_start(out=out_d[bass.ts(m, 128), cs], in_=ot)

            def emit_qtproj(qx, pool):
                qp = pool.tile([128, 512], F32, tag="b", name="qp")
                for k in range(N_KT):
                    nc.tensor.matmul(
                        qp, wqb_sb[:, k, :], xTb_sb[:, k, bass.ts(qx, 512)],
                        start=(k == 0), stop=(k == N_KT - 1),
                    )
                nc.vector.tensor_copy(QTb[:, bass.ts(qx, 512)], qp)

            def emit_extras(qt, ks):
                # deferred work, at most one small piece per chunk, placed
                # after scores(n+1) in the PE queue
                if qt == 0:
                    if 0 <= ks < 4:          # V group 1: blocks 4..7
                        emit_vblock(tr_pool, 1, ks)
                    elif 4 <= ks < 8:        # V group 2: blocks 8..11
                        emit_vblock(po_pool, 2, ks - 4)
                    elif 8 <= ks < 12:       # V group 3: blocks 12..15
                        emit_vblock(tr_pool, 3, ks - 8)
                else:
                    if 3 <= ks < 11:         # O-projection of qt-1, 8 pieces
                        emit_oproj_piece(qt - 1, ks - 3, po_pool if ks % 2 else tr_pool)
                    elif qt == 1 and ks == 12:
                        emit_qtproj(2, po_pool)
                    elif qt == 2 and ks == 12:
                        emit_qtproj(3, po_pool)

            s_cur = emit_scores(0)
            for qt in range(N_QT):
                qs = bass.ts(qt, 512)
                av0 = a0_pool.tile([128, 512], F32, tag="b", name="av0")
                av1 = a1_pool.tile([128, 512], F32, tag="b", name="av1")
                for ks in range(N_ST):
                    n = qt * N_ST + ks
                    s_next = emit_scores(n + 1) if n + 1 < N_QT * N_ST else None
                    emit_extras(qt, ks)
                    pt = pt_pool.tile([128, 2, 512], BF16, tag="pt")
                    nc.scalar.activation(
                        out=pt, in_=s_cur, func=mybir.ActivationFunctionType.Exp,
                        scale=SCALE,
                    )
                    s_cur = s_next
                    # M=65: rows 0:64 accumulate V^T @ PT, row 64 (ones
                    # column) accumulates the softmax denominators
                    nc.tensor.matmul(
                        av0[0:HD + 1, :], V_sb[:, ks, 0, :], pt[:, 0, :],
                        start=(ks == 0), stop=(ks == N_ST - 1),
                    )
                    nc.tensor.matmul(
                        av1[0:HD + 1, :], V_sb[:, ks, 1, :], pt[:, 1, :],
                        start=(ks == 0), stop=(ks == N_ST - 1),
                    )
                # normalize: s_row + reciprocal first (they gate the gpsimd
                # broadcast), then the avc stages (overlap the broadcast)
                s_row = norm_pool.tile([1, 2, 512], F32, tag="s_row")
                nc.vector.tensor_copy(s_row[0:1, 0, :], av0[64:65, :])
                nc.vector.tensor_copy(s_row[0:1, 1, :], av1[64:65, :])
                r0_sb = norm_pool.tile([1, 2, 512], F32, tag="r0")
                nc.vector.reciprocal_approx_fast(
                    out=r0_sb[0:1, 0, :], in_=s_row[0:1, 0, :]
                )
                nc.vector.reciprocal_approx_fast(
                    out=r0_sb[0:1, 1, :], in_=s_row[0:1, 1, :]
                )
                rb_sb = norm_pool.tile([64, 2, 512], F32, tag="rb")
                nc.gpsimd.partition_broadcast(
                    out_ap=rb_sb[0:64, 0, :], in_ap=r0_sb[0:1, 0, :]
                )
                nc.gpsimd.partition_broadcast(
                    out_ap=rb_sb[0:64, 1, :], in_ap=r0_sb[0:1, 1, :]
                )
                avc_sb = norm_pool.tile([64, 2, 512], F32, tag="avc")
                nc.vector.tensor_copy(avc_sb[:, 0, :], av0[0:64, :])
                nc.vector.tensor_copy(avc_sb[:, 1, :], av1[0:64, :])
                nc.vector.tensor_mul(
                    Z_sb[0:64, qs], avc_sb[:, 0, :], rb_sb[0:64, 0, :]
                )
                nc.vector.tensor_mul(
                    Z_sb[64:128, qs], avc_sb[:, 1, :], rb_sb[0:64, 1, :]
                )
            for pi in range(8):
                emit_oproj_piece(N_QT - 1, pi, po_pool if pi % 2 else tr_pool)

    nc.compile()
    return nc


_NC = None


def _get_nc():
    global _NC
    if _NC is None:
        _NC = build_nc()
    return _NC


def _tiled(a):
    """[512, N] -> [128, 4, N] bf16 with contiguous per-partition rows."""
    n = a.shape[1]
    return np.ascontiguousarray(
        a.reshape(N_KT, 128, n).transpose(1, 0, 2)
    ).astype(ml_dtypes.bfloat16)


def make_in_maps(x, w_qkv, w_o):
    x = np.asarray(x, dtype=np.float32)
    w_qkv = np.asarray(w_qkv, dtype=np.float32)
    w_o = np.asarray(w_o, dtype=np.float32)
    in_maps = []
    xTs = [_tiled(np.ascontiguousarray(x[b].T)) for b in range(B)]
    for c in range(8):
        b, g = c // 4, c % 4
        cols = slice(2 * g * HD, (2 * g + 2) * HD)
        in_maps.append({
            "xTb": xTs[b],
            "wqb": _tiled(w_qkv[:, :EMBED][:, cols]),
            "wkb": _tiled(w_qkv[:, EMBED:2 * EMBED][:, cols]),
            "wvb": _tiled(w_qkv[:, 2 * EMBED:][:, cols]),
            "wo": np.ascontiguousarray(w_o[cols, :]),
        })
    return in_maps


def combine(results, b_o):
    partials = np.stack(
        [np.asarray(r["out"]).astype(np.float32) for r in results]
    )  # [8, S, EMBED]
    out = partials.reshape(B, 4, S, EMBED).sum(axis=1)
    return (out + np.asarray(b_o, dtype=np.float32)).astype(np.float32)


def kernel(x, w_qkv, w_o, b_o):
    nc = _get_nc()
    res = run_bass_kernel_spmd(nc, make_in_maps(x, w_qkv, w_o), core_ids=list(range(8)))
    return combine(res.results, b_o)
